# revision 30
# baseline (speedup 1.0000x reference)
"""Trainium2 Bass/Tile kernel for nn_AttnBlock_29712583753795.

Per sample (B=16, C=512, H=W=64, n=4096):
    xn  = groupnorm(x; 16 groups, w1, b1)
    kv  = kv_w @ xn + kv_b                  (1x1 conv -> [2C, n])
    k, v = split(kv)
    q   = softmax_c(k) * C^-0.5
    k   = softmax_n(k)
    ctx = k @ v.T                           [C, C]
    o2  = ctx.T @ q                         [C, n]
    out = out_w @ gelu(groupnorm(o2; w2, b2)) + out_b
    return xn + out

Sharding: pure data-parallel over batch. 2 samples per NeuronCore, 8 cores.

Key algebraic folds (avoid all transposes of the big [C, n] tensors):
  * softmax_n(k) is invariant to the k-bias (constant along n) -> the kv-pass
    that feeds the context matmul needs NO bias at all.
  * context row-normalization (1/R[d]) and the v-bias fold into cheap
    per-partition / small-tile ops on the [C, C] context matrix:
        ctx_final = ctx_raw / R[d] + vb[e]
  * the q-softmax normalizer S[n] rides the attention matmul as an extra
    ones-vector matmul; 1/S is broadcast across partitions with a K=1 matmul.
  * k is computed twice (once as k^T [n,C] for the context contraction over n,
    once as k [C,n] for the attention contraction over d) - cheaper than
    materializing an 8MB transpose.
All big matmuls run as float32r (fp32 data, relaxed PE mode, 1 cyc/row at
free-dim >= 256). The BIR verifier requires every f32r matmul operand to be
written by an f32r-rounding compute op, so weights are staged through a DVE
copy and activations write through f32r-bitcast output APs.

Schedule shape (per core, 2 samples):
  GN1 (streamed stats; next sample's stats run during this sample's phases)
  -> phase 1: kv^T matmuls + exp + context accumulation (+R row rider)
  -> phase 2: k matmuls + exp + attention out (+S rider), out2 spilled to
     DRAM in bf16 -> GN2 stats -> phase 3: gelu+proj+fused bias/residual.
rsqrt for the group norms is computed on DVE (bit-hack + 3 Newton steps) to
avoid ACT sqrt/ln table loads thrashing against the exp/gelu tables.
Cost-model (instruction_cost_v2 TimelineSim) exec: ~504 us/core, PE 72% busy.
"""

import sys

for _p in ("/opt/trn_rl_repo",):
    if _p not in sys.path:
        sys.path.insert(0, _p)

import numpy as np

import concourse.bass as bass
import concourse.tile as tile
from concourse import bacc, mybir
from concourse.bass_utils import run_bass_kernel_spmd

F32 = mybir.dt.float32
F32R = mybir.dt.float32r
BF16 = mybir.dt.bfloat16
I32 = mybir.dt.int32
AX = mybir.AxisListType
OP = mybir.AluOpType
AF = mybir.ActivationFunctionType

N_CORES = 8
B, C, H, W = 16, 512, 64, 64
N = H * W                      # 4096 spatial
BPC = B // N_CORES             # 2 samples per core
P = 128                        # partitions
CT = C // P                    # 4 channel tiles
NT = N // P                    # 32 n-tiles (phase 1)
NCH = N // 512                 # 8 n-chunks of 512 (phases 2/3)
GROUPS = 16
GSIZE = C // GROUPS            # 32 channels per group
GN_COUNT = float(GSIZE * N)    # 131072 elements per group
EPS = 1e-5
QINV = float(np.sqrt(float(C)))  # 1/q_scale


def _r(ap):
    return ap.bitcast(F32R)


def build_program(gelu: bool = True, reps: int = 1):
    """Build the per-core Bass program (identical on all 8 cores)."""
    nc = bacc.Bacc("TRN2", target_bir_lowering=False, debug=False,
                   num_devices=N_CORES)

    x_d = nc.dram_tensor("x", [BPC * C, N], F32, kind="ExternalInput").ap()
    kvw_d = nc.dram_tensor("kvwT", [C, 2 * C], F32, kind="ExternalInput").ap()
    outw_d = nc.dram_tensor("outwT", [C, C], F32, kind="ExternalInput").ap()
    prm_d = nc.dram_tensor("prm", [6, CT, P], F32, kind="ExternalInput").ap()
    vb_d = nc.dram_tensor("vb", [1, C], F32, kind="ExternalInput").ap()
    gm_d = nc.dram_tensor("gmat", [P, 4], F32, kind="ExternalInput").ap()
    gmT_d = nc.dram_tensor("gmatT", [4, P], F32, kind="ExternalInput").ap()
    out_d = nc.dram_tensor("out", [BPC * C, N], F32, kind="ExternalOutput").ap()

    gelu_f = AF.Gelu if gelu else AF.Identity

    with tile.TileContext(nc) as tc:
        from contextlib import ExitStack
        with ExitStack() as ctx:
            E = ctx.enter_context
            const = E(tc.tile_pool(name="const", bufs=1))
            x_pool = E(tc.tile_pool(name="x", bufs=5))
            xraw_pool = E(tc.tile_pool(name="xraw", bufs=2))
            o2rd_pool = E(tc.tile_pool(name="o2rd", bufs=5))
            ctxsb_pool = E(tc.tile_pool(name="ctxsb", bufs=4))
            ekt_pool = E(tc.tile_pool(name="ekt", bufs=2))
            vt_pool = E(tc.tile_pool(name="vt", bufs=2))
            ek2_pool = E(tc.tile_pool(name="ek2", bufs=3))
            g_pool = E(tc.tile_pool(name="g", bufs=7))
            xap_pool = E(tc.tile_pool(name="xap", bufs=4))
            bcs_pool = E(tc.tile_pool(name="bcs", bufs=1))
            outsb_pool = E(tc.tile_pool(name="outsb", bufs=3))
            dump_pool = E(tc.tile_pool(name="dump", bufs=1))
            stat_pool = E(tc.tile_pool(name="stat", bufs=4))
            ab_pool = E(tc.tile_pool(name="ab", bufs=10))
            small_pool = E(tc.tile_pool(name="small", bufs=1))
            dram_pool = E(tc.tile_pool(name="drsc", bufs=2, space="DRAM"))

            # PSUM: 8 banks total, statically reserved -> share 3 pools
            # across phases via common tags (each slot = one [128,512] bank).
            quad_ps = E(tc.tile_pool(name="quad_ps", bufs=5, space="PSUM"))
            tri_ps = E(tc.tile_pool(name="tri_ps", bufs=2, space="PSUM"))
            row_ps = E(tc.tile_pool(name="row_ps", bufs=1, space="PSUM"))

            # -------- prefetch sample-0 x before weight staging ---------
            xr_prefetch = {}
            for ct in range(CT):
                xr = xraw_pool.tile([P, N], F32, name="xr", tag="xr")
                nc.sync.dma_start(xr[:, :N // 2], x_d[ct * P:(ct + 1) * P, :N // 2])
                nc.sync.dma_start(xr[:, N // 2:], x_d[ct * P:(ct + 1) * P, N // 2:])
                xr_prefetch[(0, ct)] = xr

            # ---------------- constants ----------------
            # f32r matmul operands must be written by a rounding instruction:
            # DMA weights into staging, DVE-copy into the const tiles as f32r.
            def stage_round(dst_slice, src_slice, rows=P):
                # stage through the g pool (idle at startup, 8 slots deep)
                stg = g_pool.tile([P, 512], F32, name="stg", tag="g")
                nc.sync.dma_start(stg[:rows, :src_slice.shape[-1]], src_slice)
                nc.vector.tensor_copy(_r(dst_slice),
                                      stg[:rows, :src_slice.shape[-1]])

            # staging order = order of first use: k-halves (phase 1 kps),
            # v-halves (vps), vb (ctx drain), outw (phase 3).
            kvw_sb = const.tile([P, CT * 2 * C], F32)    # [128, 4096]
            for h in range(2):
                for ct in range(CT):
                    stage_round(
                        kvw_sb[:, ct * 2 * C + h * 512: ct * 2 * C + (h + 1) * 512],
                        kvw_d[ct * P:(ct + 1) * P, h * 512:(h + 1) * 512])
            # per-channel params, one [128, CT] tile each: w1,b1,kb,w2,b2,ob
            pcols = []
            for idx in range(6):
                t = const.tile([P, CT], F32, name=f"prm{idx}", tag=f"prm{idx}")
                nc.sync.dma_start(t, prm_d[idx].rearrange("t p -> p t"))
                pcols.append(t)
            w1c, b1c, kbc, w2c, b2c, obc = pcols
            gm = const.tile([P, 4], F32)
            nc.sync.dma_start(gm, gm_d)
            gmT = const.tile([4, P], F32)
            nc.sync.dma_start(gmT, gmT_d)
            vb_row = const.tile([1, C], F32)
            stage_round(vb_row, vb_d, rows=1)
            outw_sb = const.tile([P, CT * C], F32)       # [128, 2048]
            for et in range(CT):
                stage_round(outw_sb[:, et * C:(et + 1) * C],
                            outw_d[et * P:(et + 1) * P, :])
            ones_col = const.tile([P, 1], F32)
            qinv_col = const.tile([P, 1], F32)
            ones_row = const.tile([1, P], F32)
            mset = g_pool.tile([P, 512], F32, name="mset", tag="g")
            nc.vector.memset(mset[:, 0:P], 1.0)
            nc.vector.tensor_copy(_r(ones_col), mset[:, 0:1])
            nc.vector.tensor_copy(_r(ones_row), mset[0:1, 0:P])
            nc.vector.memset(mset[:, 1:2], QINV)
            nc.vector.tensor_copy(_r(qinv_col), mset[:, 1:2])
            # vb broadcast to all partitions via K=1 matmul
            vb_bcast = const.tile([P, C], F32)
            bps0 = row_ps.tile([P, C], F32, name="bps0", tag="row")
            nc.tensor.matmul(bps0, _r(ones_row), _r(vb_row), start=True, stop=True)
            nc.scalar.copy(vb_bcast, bps0)

            def gn_params(stats2, wcol, bcol):
                """stats2: [128,2] SBUF (sum, sumsq) per channel ->
                ab [128,2] tile: A = w*rstd, B = b - mu*A."""
                gps = tri_ps.tile([4, 2], F32, name="gps", tag="tri")
                nc.tensor.matmul(gps, gm, stats2, start=True, stop=True)
                gmn = stat_pool.tile([4, 2], F32)
                nc.vector.tensor_scalar_mul(gmn, gps, 1.0 / GN_COUNT)
                musq = stat_pool.tile([4, 1], F32)
                nc.vector.tensor_mul(musq, gmn[:, 0:1], gmn[:, 0:1])
                murstd = stat_pool.tile([4, 2], F32)
                varv = stat_pool.tile([4, 1], F32)
                nc.vector.tensor_sub(varv, gmn[:, 1:2], musq)
                nc.vector.tensor_scalar_add(varv, varv, EPS)
                # rsqrt on DVE only (bit-hack seed + 3 Newton steps): avoids
                # ACT sqrt/ln table loads that thrash against the exp table.
                yt = stat_pool.tile([4, 1], F32)
                vi = yt.bitcast(I32)
                nc.vector.tensor_scalar(vi, varv.bitcast(I32), 1, None,
                                        op0=OP.arith_shift_right)
                nc.vector.tensor_scalar(vi, vi, -1, 0x5F3759DF,
                                        op0=OP.mult, op1=OP.add)
                for it in range(3):
                    t2 = stat_pool.tile([4, 1], F32, name=f"t2_{it}", tag="t2")
                    nc.vector.tensor_mul(t2, yt, yt)
                    nc.vector.tensor_mul(t2, t2, varv)
                    nc.vector.tensor_scalar(t2, t2, -0.5, 1.5,
                                            op0=OP.mult, op1=OP.add)
                    nc.vector.tensor_mul(
                        murstd[:, 1:2] if it == 2 else yt,
                        yt, t2)
                nc.vector.tensor_copy(murstd[:, 0:1], gmn[:, 0:1])
                cps = tri_ps.tile([P, 2], F32, name="cps", tag="tri")
                nc.tensor.matmul(cps, gmT, murstd, start=True, stop=True)
                ab = ab_pool.tile([P, 2], F32)
                nc.vector.tensor_mul(ab[:, 0:1], wcol, cps[:, 1:2])
                tmpm = stat_pool.tile([P, 1], F32)
                nc.vector.tensor_mul(tmpm, cps[:, 0:1], ab[:, 0:1])
                nc.vector.tensor_sub(ab[:, 1:2], bcol, tmpm)
                return ab

            def gn1_resident(s):
                """Stats + apply per ctile from resident full-x tiles."""
                xn = []
                for ct in range(CT):
                    rows = slice(s * C + ct * P, s * C + (ct + 1) * P)
                    xr = xr_prefetch.pop((s, ct), None)
                    if xr is None:
                        xr = xraw_pool.tile([P, N], F32, name="xr", tag="xr")
                        nc.sync.dma_start(xr[:, :N // 2], x_d[rows, :N // 2])
                        nc.sync.dma_start(xr[:, N // 2:], x_d[rows, N // 2:])
                    sm8 = stat_pool.tile([P, 8], F32)
                    sq8 = stat_pool.tile([P, 8], F32)
                    for j in range(NCH):
                        sl = xr[:, j * 512:(j + 1) * 512]
                        dmp = dump_pool.tile([P, 512], F32)
                        nc.scalar.activation(dmp, sl, AF.Square,
                                             accum_out=sq8[:, j:j + 1])
                        nc.vector.reduce_sum(sm8[:, j:j + 1], sl, axis=AX.X)
                    st2 = stat_pool.tile([P, 2], F32)
                    nc.vector.reduce_sum(st2[:, 0:1], sm8, axis=AX.X)
                    nc.vector.reduce_sum(st2[:, 1:2], sq8, axis=AX.X)
                    ab1 = gn_params(st2, w1c[:, ct:ct + 1], b1c[:, ct:ct + 1])
                    xnt = x_pool.tile([P, N], F32, name="xnt", tag="xnt")
                    nc.vector.tensor_scalar(_r(xnt), xr, ab1[:, 0:1],
                                            ab1[:, 1:2], op0=OP.mult,
                                            op1=OP.add)
                    xn.append(xnt)
                return xn

            def gn1_stats_streaming(s):
                """Stats from streamed chunks (emitted early, runs during the
                previous sample's phases; x is re-read at apply time)."""
                abs_ = []
                for ct in range(CT):
                    rows = slice(s * C + ct * P, s * C + (ct + 1) * P)
                    sm8 = stat_pool.tile([P, 8], F32)
                    sq8 = stat_pool.tile([P, 8], F32)
                    for j in range(NCH):
                        xc = xap_pool.tile([P, 512], F32, name="xc", tag="xap")
                        nc.sync.dma_start(xc, x_d[rows, j * 512:(j + 1) * 512])
                        dmp = dump_pool.tile([P, 512], F32)
                        nc.scalar.activation(dmp, xc, AF.Square,
                                             accum_out=sq8[:, j:j + 1])
                        nc.vector.reduce_sum(sm8[:, j:j + 1], xc, axis=AX.X)
                    st2 = stat_pool.tile([P, 2], F32)
                    nc.vector.reduce_sum(st2[:, 0:1], sm8, axis=AX.X)
                    nc.vector.reduce_sum(st2[:, 1:2], sq8, axis=AX.X)
                    abs_.append(gn_params(st2, w1c[:, ct:ct + 1],
                                          b1c[:, ct:ct + 1]))
                return abs_

            seq = [s for _ in range(reps) for s in range(BPC)]
            pending_stats = {}
            for idx, s in enumerate(seq):
                row0 = s * C
                # ============ GroupNorm 1 apply -> xn tiles ================
                if idx == 0:
                    xn = gn1_resident(s)
                else:
                    abs_ = pending_stats.pop(idx)
                    xn = []
                    for ct in range(CT):
                        xn.append(x_pool.tile([P, N], F32, name="xnt",
                                              tag="xnt"))
                    # chunk-major applies: phase 1 unblocks after the first
                    # column of every ctile
                    for j in range(NCH):
                        for ct in range(CT):
                            rows = slice(row0 + ct * P, row0 + (ct + 1) * P)
                            xc = xap_pool.tile([P, 512], F32, name="xc",
                                               tag="xap")
                            nc.sync.dma_start(
                                xc, x_d[rows, j * 512:(j + 1) * 512])
                            nc.vector.tensor_scalar(
                                _r(xn[ct][:, j * 512:(j + 1) * 512]), xc,
                                abs_[ct][:, 0:1], abs_[ct][:, 1:2],
                                op0=OP.mult, op1=OP.add)

                # ================= Phase 1: kv^T pass + context =============
                ctx_acc = [quad_ps.tile([P, C], F32, name="ctx_acc", tag="quad") for _ in range(CT)]
                r_row = row_ps.tile([1, C], F32, name="r_row", tag="row")

                def emit_ctx(ekt, vt, nt):
                    nc.tensor.matmul(r_row, _r(ones_col), _r(ekt),
                                     start=(nt == 0), stop=(nt == NT - 1))
                    for dt in range(CT):
                        nc.tensor.matmul(ctx_acc[dt],
                                         _r(ekt[:, dt * P:(dt + 1) * P]), _r(vt),
                                         start=(nt == 0), stop=(nt == NT - 1))

                prev = None
                for nt in range(NT):
                    kps = tri_ps.tile([P, 512], F32, name="kps", tag="tri")
                    for ct in range(CT):
                        nc.tensor.matmul(
                            kps, _r(xn[ct][:, nt * P:(nt + 1) * P]),
                            _r(kvw_sb[:, ct * 2 * C: ct * 2 * C + 512]),
                            start=(ct == 0), stop=(ct == CT - 1))
                    vps = tri_ps.tile([P, 512], F32, name="vps", tag="tri")
                    for ct in range(CT):
                        nc.tensor.matmul(
                            vps, _r(xn[ct][:, nt * P:(nt + 1) * P]),
                            _r(kvw_sb[:, ct * 2 * C + 512: (ct + 1) * 2 * C]),
                            start=(ct == 0), stop=(ct == CT - 1))
                    ekt = ekt_pool.tile([P, 512], F32)
                    nc.scalar.activation(_r(ekt), kps, AF.Exp)  # k-bias cancels
                    vt = vt_pool.tile([P, 512], F32)
                    nc.vector.tensor_copy(_r(vt), vps)      # v-bias folded later
                    if prev is not None:
                        emit_ctx(*prev)
                    prev = (ekt, vt, nt)
                emit_ctx(*prev)

                # R: [1,512] row -> per-partition columns via DRAM bounce
                r_sb = small_pool.tile([1, C], F32, name="r_sb", tag="rcs")
                nc.scalar.copy(r_sb, r_row)
                rb = dram_pool.tile([1, C], F32)
                nc.sync.dma_start(rb, r_sb)
                rcol = small_pool.tile([P, CT], F32)
                nc.sync.dma_start(rcol, rb.rearrange("a (t p) -> (a p) t", p=P))
                rcp = small_pool.tile([P, CT], F32)
                nc.vector.reciprocal(rcp, rcol)
                ctx_sb = []
                for dt in range(CT):
                    t = ctxsb_pool.tile([P, C], F32, name="ctx_sb", tag="ctx_sb")
                    # ctx/R + vb in one DVE op
                    nc.vector.scalar_tensor_tensor(
                        _r(t), ctx_acc[dt], rcp[:, dt:dt + 1], vb_bcast,
                        op0=OP.mult, op1=OP.add)
                    ctx_sb.append(t)
                # next iteration's GN1 stats: emitted here so they run during
                # this sample's phase-2/3 window
                if idx + 1 < len(seq):
                    pending_stats[idx + 1] = gn1_stats_streaming(seq[idx + 1])

                # ================= Phase 2: k pass + attention out ==========
                o2dram = dram_pool.tile([C, N], BF16, name="o2dram", tag="o2dram")
                s2_8 = [stat_pool.tile([P, 8], F32, name="s2_8", tag="s2_8") for _ in range(CT)]
                q2_8 = [stat_pool.tile([P, 8], F32, name="q2_8", tag="q2_8") for _ in range(CT)]
                o2ps = {}
                sps = {}

                def emit_attn(j, dt, ek2):
                    nc.tensor.matmul(sps[j], _r(qinv_col), _r(ek2),
                                     start=(dt == 0), stop=(dt == CT - 1))
                    for et in range(CT):
                        nc.tensor.matmul(o2ps[j][et],
                                         _r(ctx_sb[dt][:, et * P:(et + 1) * P]),
                                         _r(ek2),
                                         start=(dt == 0), stop=(dt == CT - 1))
                    if dt == CT - 1:
                        # drain chunk j: 1/S broadcast, scale, GN2 stats
                        rcs = small_pool.tile([1, 512], F32, name="rcs", tag="rcs")
                        with nc.allow_low_precision(reason="f32r rounding for matmul rhs"):
                            nc.vector.reciprocal(_r(rcs), sps[j][0:1, :])
                        bps = row_ps.tile([P, 512], F32, name="bps", tag="row")
                        nc.tensor.matmul(bps, _r(ones_row), _r(rcs),
                                         start=True, stop=True)
                        bcs = bcs_pool.tile([P, 512], F32)
                        nc.scalar.copy(bcs, bps)
                        for et in range(CT):
                            stg2 = outsb_pool.tile([P, 512], BF16, name="stg2",
                                                   tag="outsb")
                            nc.vector.tensor_mul(stg2, o2ps[j][et], bcs)
                            dmp = dump_pool.tile([P, 512], F32)
                            nc.scalar.activation(dmp, stg2, AF.Square,
                                                 accum_out=q2_8[et][:, j:j + 1])
                            nc.vector.reduce_sum(s2_8[et][:, j:j + 1], stg2,
                                                 axis=AX.X)
                            nc.sync.dma_start(
                                o2dram[et * P:(et + 1) * P,
                                       j * 512:(j + 1) * 512], stg2)
                        del o2ps[j], sps[j]

                pending2 = []
                for j in range(NCH):
                    o2ps[j] = [quad_ps.tile([P, 512], F32, name="o2ps", tag="quad") for _ in range(CT)]
                    sps[j] = row_ps.tile([1, 512], F32, name="sps", tag="row")
                    for dt in range(CT):
                        k2 = tri_ps.tile([P, 512], F32, name="k2", tag="tri")
                        for ct in range(CT):
                            nc.tensor.matmul(
                                k2,
                                _r(kvw_sb[:, ct * 2 * C + dt * P:
                                          ct * 2 * C + (dt + 1) * P]),
                                _r(xn[ct][:, j * 512:(j + 1) * 512]),
                                start=(ct == 0), stop=(ct == CT - 1))
                        ek2 = ek2_pool.tile([P, 512], F32, name="ek2", tag="ek2")
                        nc.scalar.activation(_r(ek2), k2, AF.Exp,
                                             bias=kbc[:, dt:dt + 1])
                        pending2.append((j, dt, ek2))
                        if len(pending2) > 2:
                            emit_attn(*pending2.pop(0))
                for p2 in pending2:
                    emit_attn(*p2)

                # ================= GroupNorm 2 params =======================
                ab2 = []
                for et in range(CT):
                    st2 = stat_pool.tile([P, 2], F32)
                    nc.vector.reduce_sum(st2[:, 0:1], s2_8[et], axis=AX.X)
                    nc.vector.reduce_sum(st2[:, 1:2], q2_8[et], axis=AX.X)
                    ab2.append(gn_params(st2, w2c[:, et:et + 1], b2c[:, et:et + 1]))

                # ================= Phase 3: gelu + proj + residual ==========
                def emit_proj(j, gts):
                    for ot in range(CT):
                        o3 = quad_ps.tile([P, 512], F32, name="o3", tag="quad")
                        for et in range(CT):
                            nc.tensor.matmul(
                                o3,
                                _r(outw_sb[:, et * C + ot * P: et * C + (ot + 1) * P]),
                                _r(gts[et]),
                                start=(et == 0), stop=(et == CT - 1))
                        ob_sb = outsb_pool.tile([P, 512], F32, name="ob_sb",
                                                 tag="outsb")
                        # (o3 + out_b) + xn in one DVE op
                        nc.vector.scalar_tensor_tensor(
                            ob_sb, o3, obc[:, ot:ot + 1],
                            _r(xn[ot][:, j * 512:(j + 1) * 512]),
                            op0=OP.add, op1=OP.add)
                        nc.sync.dma_start(
                            out_d[row0 + ot * P: row0 + (ot + 1) * P,
                                  j * 512:(j + 1) * 512], ob_sb)

                prev3 = None
                for j in range(NCH):
                    gts = []
                    for et in range(CT):
                        rd = o2rd_pool.tile([P, 512], BF16, name="rd", tag="rd")
                        nc.sync.dma_start(
                            rd, o2dram[et * P:(et + 1) * P,
                                       j * 512:(j + 1) * 512])
                        g = g_pool.tile([P, 512], F32, name="g", tag="g")
                        nc.scalar.activation(_r(g), rd,
                                             gelu_f, bias=ab2[et][:, 1:2],
                                             scale=ab2[et][:, 0:1])
                        gts.append(g)
                    if prev3 is not None:
                        emit_proj(*prev3)
                    prev3 = (j, gts)
                emit_proj(*prev3)

    nc.compile()
    return nc


def prep_inputs(inputs):
    """Host-side prep: shard x over batch, pre-transpose/pack weights."""
    x = np.ascontiguousarray(np.asarray(inputs["x"], dtype=np.float32))
    kv_w = np.asarray(inputs["kv_w"], dtype=np.float32)
    kv_b = np.asarray(inputs["kv_b"], dtype=np.float32)
    out_w = np.asarray(inputs["out_w"], dtype=np.float32)
    out_b = np.asarray(inputs["out_b"], dtype=np.float32)
    w1 = np.asarray(inputs["norm1_w"], dtype=np.float32)
    b1 = np.asarray(inputs["norm1_b"], dtype=np.float32)
    w2 = np.asarray(inputs["norm2_w"], dtype=np.float32)
    b2 = np.asarray(inputs["norm2_b"], dtype=np.float32)

    kvwT = np.ascontiguousarray(kv_w.T)                 # [C, 2C]
    outwT = np.ascontiguousarray(out_w.T)               # [C, C]
    kb = kv_b[:C]
    vb = np.ascontiguousarray(kv_b[C:]).reshape(1, C)
    prm = np.stack([w1, b1, kb, w2, b2, out_b]).reshape(6, CT, P)
    prm = np.ascontiguousarray(prm)
    gmat = np.zeros((P, 4), np.float32)
    for p in range(P):
        gmat[p, p // GSIZE] = 1.0
    gmatT = np.ascontiguousarray(gmat.T)

    xs = x.reshape(B, C, N)
    in_maps = []
    for i in range(N_CORES):
        shard = np.ascontiguousarray(
            xs[i * BPC:(i + 1) * BPC].reshape(BPC * C, N))
        in_maps.append({
            "x": shard, "kvwT": kvwT, "outwT": outwT, "prm": prm,
            "vb": vb, "gmat": gmat, "gmatT": gmatT,
        })
    return in_maps


_NC_CACHE = {}


def get_program(gelu: bool = True, reps: int = 1):
    key = (bool(gelu), reps)
    if key not in _NC_CACHE:
        _NC_CACHE[key] = build_program(gelu=key[0], reps=reps)
    return _NC_CACHE[key]


def run(inputs, trace: bool = False, gelu: bool = True, reps: int = 1):
    """Run on 8 cores; returns (full_output [16,512,64,64], BassKernelResults)."""
    nc = get_program(gelu=gelu, reps=reps)
    in_maps = prep_inputs(inputs)
    res = run_bass_kernel_spmd(nc, in_maps, core_ids=list(range(N_CORES)),
                               trace=trace)
    full = np.empty((B, C, N), np.float32)
    for i in range(N_CORES):
        full[i * BPC:(i + 1) * BPC] = res.results[i]["out"].reshape(BPC, C, N)
    return full.reshape(B, C, H, W), res


def kernel(**inputs) -> np.ndarray:
    out, _ = run(inputs, trace=False, gelu=True)
    return out


# revision 33
# speedup vs baseline: 1.0228x; 1.0228x over previous
"""Trainium2 Bass/Tile kernel for nn_AttnBlock_29712583753795.

Per sample (B=16, C=512, H=W=64, n=4096):
    xn  = groupnorm(x; 16 groups, w1, b1)
    kv  = kv_w @ xn + kv_b                  (1x1 conv -> [2C, n])
    k, v = split(kv)
    q   = softmax_c(k) * C^-0.5
    k   = softmax_n(k)
    ctx = k @ v.T                           [C, C]
    o2  = ctx.T @ q                         [C, n]
    out = out_w @ gelu(groupnorm(o2; w2, b2)) + out_b
    return xn + out

Sharding: pure data-parallel over batch. 2 samples per NeuronCore, 8 cores.

Key algebraic folds (avoid all transposes of the big [C, n] tensors):
  * softmax_n(k) is invariant to the k-bias (constant along n) -> the kv-pass
    that feeds the context matmul needs NO bias at all.
  * context row-normalization (1/R[d]) and the v-bias fold into cheap
    per-partition / small-tile ops on the [C, C] context matrix:
        ctx_final = ctx_raw / R[d] + vb[e]
  * the q-softmax normalizer S[n] rides the attention matmul as an extra
    ones-vector matmul; 1/S is broadcast across partitions with a K=1 matmul.
  * k is computed twice (once as k^T [n,C] for the context contraction over n,
    once as k [C,n] for the attention contraction over d) - cheaper than
    materializing an 8MB transpose.
All big matmuls run as float32r (fp32 data, relaxed PE mode, 1 cyc/row at
free-dim >= 256). The BIR verifier requires every f32r matmul operand to be
written by an f32r-rounding compute op, so weights are staged through a DVE
copy and activations write through f32r-bitcast output APs.

Schedule shape (per core, 2 samples):
  GN1 (streamed stats; next sample's stats run during this sample's phases)
  -> phase 1: kv^T matmuls + exp + context accumulation (+R row rider)
  -> phase 2: k matmuls + exp + attention out (+S rider), out2 spilled to
     DRAM in bf16 -> GN2 stats -> phase 3: gelu+proj+fused bias/residual.
rsqrt for the group norms is computed on DVE (bit-hack + 3 Newton steps) to
avoid ACT sqrt/ln table loads thrashing against the exp/gelu tables.
Cost-model (instruction_cost_v2 TimelineSim) exec: ~504 us/core, PE 72% busy.
"""

import sys

for _p in ("/opt/trn_rl_repo",):
    if _p not in sys.path:
        sys.path.insert(0, _p)

import numpy as np

import concourse.bass as bass
import concourse.tile as tile
from concourse import bacc, mybir
from concourse.bass_utils import run_bass_kernel_spmd

F32 = mybir.dt.float32
F32R = mybir.dt.float32r
BF16 = mybir.dt.bfloat16
I32 = mybir.dt.int32
AX = mybir.AxisListType
OP = mybir.AluOpType
AF = mybir.ActivationFunctionType

N_CORES = 8
B, C, H, W = 16, 512, 64, 64
N = H * W                      # 4096 spatial
BPC = B // N_CORES             # 2 samples per core
P = 128                        # partitions
CT = C // P                    # 4 channel tiles
NT = N // P                    # 32 n-tiles (phase 1)
NCH = N // 512                 # 8 n-chunks of 512 (phases 2/3)
GROUPS = 16
GSIZE = C // GROUPS            # 32 channels per group
GN_COUNT = float(GSIZE * N)    # 131072 elements per group
EPS = 1e-5
QINV = float(np.sqrt(float(C)))  # 1/q_scale


def _r(ap):
    return ap.bitcast(F32R)


def build_program(gelu: bool = True, reps: int = 1):
    """Build the per-core Bass program (identical on all 8 cores)."""
    nc = bacc.Bacc("TRN2", target_bir_lowering=False, debug=False,
                   num_devices=N_CORES)

    x_d = nc.dram_tensor("x", [BPC * C, N], F32, kind="ExternalInput").ap()
    kvw_d = nc.dram_tensor("kvwT", [C, 2 * C], F32, kind="ExternalInput").ap()
    outw_d = nc.dram_tensor("outwT", [C, C], F32, kind="ExternalInput").ap()
    prm_d = nc.dram_tensor("prm", [6, CT, P], F32, kind="ExternalInput").ap()
    vb_d = nc.dram_tensor("vb", [1, C], F32, kind="ExternalInput").ap()
    gm_d = nc.dram_tensor("gmat", [P, 4], F32, kind="ExternalInput").ap()
    gmT_d = nc.dram_tensor("gmatT", [4, P], F32, kind="ExternalInput").ap()
    out_d = nc.dram_tensor("out", [BPC * C, N], F32, kind="ExternalOutput").ap()

    gelu_f = AF.Gelu if gelu else AF.Identity

    with tile.TileContext(nc) as tc:
        from contextlib import ExitStack
        with ExitStack() as ctx:
            E = ctx.enter_context
            const = E(tc.tile_pool(name="const", bufs=1))
            x_pool = E(tc.tile_pool(name="x", bufs=6))
            o2rd_pool = E(tc.tile_pool(name="o2rd", bufs=5))
            ctxsb_pool = E(tc.tile_pool(name="ctxsb", bufs=4))
            ekt_pool = E(tc.tile_pool(name="ekt", bufs=3))
            vt_pool = E(tc.tile_pool(name="vt", bufs=3))
            ek2_pool = E(tc.tile_pool(name="ek2", bufs=3))
            g_pool = E(tc.tile_pool(name="g", bufs=7))
            xap_pool = E(tc.tile_pool(name="xap", bufs=4))
            bcs_pool = E(tc.tile_pool(name="bcs", bufs=1))
            outsb_pool = E(tc.tile_pool(name="outsb", bufs=4))
            dump_pool = E(tc.tile_pool(name="dump", bufs=2))
            stat_pool = E(tc.tile_pool(name="stat", bufs=4))
            ab_pool = E(tc.tile_pool(name="ab", bufs=10))
            small_pool = E(tc.tile_pool(name="small", bufs=1))
            dram_pool = E(tc.tile_pool(name="drsc", bufs=2, space="DRAM"))

            # PSUM: 8 banks total, statically reserved -> share 3 pools
            # across phases via common tags (each slot = one [128,512] bank).
            quad_ps = E(tc.tile_pool(name="quad_ps", bufs=5, space="PSUM"))
            tri_ps = E(tc.tile_pool(name="tri_ps", bufs=2, space="PSUM"))
            row_ps = E(tc.tile_pool(name="row_ps", bufs=1, space="PSUM"))

            # ---------------- constants ----------------
            # f32r matmul operands must be written by a rounding instruction:
            # DMA weights into staging, DVE-copy into the const tiles as f32r.
            def stage_round(dst_slice, src_slice, rows=P):
                # stage through the g pool (idle at startup, 8 slots deep)
                stg = g_pool.tile([P, 512], F32, name="stg", tag="g")
                nc.sync.dma_start(stg[:rows, :src_slice.shape[-1]], src_slice)
                nc.vector.tensor_copy(_r(dst_slice),
                                      stg[:rows, :src_slice.shape[-1]])

            kvw_sb = const.tile([P, CT * 2 * C], F32)    # [128, 4096]
            # per-channel params, one [128, CT] tile each: w1,b1,kb,w2,b2,ob
            pcols = []
            for idx in range(6):
                t = const.tile([P, CT], F32, name=f"prm{idx}", tag=f"prm{idx}")
                nc.sync.dma_start(t, prm_d[idx].rearrange("t p -> p t"))
                pcols.append(t)
            w1c, b1c, kbc, w2c, b2c, obc = pcols
            gm = const.tile([P, 4], F32)
            nc.sync.dma_start(gm, gm_d)
            gmT = const.tile([4, P], F32)
            nc.sync.dma_start(gmT, gmT_d)
            vb_row = const.tile([1, C], F32)
            outw_sb = const.tile([P, CT * C], F32)       # [128, 2048]
            ones_col = const.tile([P, 1], F32)
            qinv_col = const.tile([P, 1], F32)
            ones_row = const.tile([1, P], F32)
            mset = g_pool.tile([P, 512], F32, name="mset", tag="g")
            nc.vector.memset(mset[:, 0:P], 1.0)
            nc.vector.tensor_copy(_r(ones_col), mset[:, 0:1])
            nc.vector.tensor_copy(_r(ones_row), mset[0:1, 0:P])
            nc.vector.memset(mset[:, 1:2], QINV)
            nc.vector.tensor_copy(_r(qinv_col), mset[:, 1:2])
            # vb broadcast to all partitions via K=1 matmul
            vb_bcast = const.tile([P, C], F32)
            bps0 = row_ps.tile([P, C], F32, name="bps0", tag="row")
            nc.tensor.matmul(bps0, _r(ones_row), _r(vb_row), start=True, stop=True)
            nc.scalar.copy(vb_bcast, bps0)

            def gn_params(stats2, wcol, bcol):
                """stats2: [128,2] SBUF (sum, sumsq) per channel ->
                ab [128,2] tile: A = w*rstd, B = b - mu*A."""
                gps = tri_ps.tile([4, 2], F32, name="gps", tag="tri")
                nc.tensor.matmul(gps, gm, stats2, start=True, stop=True)
                gmn = stat_pool.tile([4, 2], F32)
                nc.vector.tensor_scalar_mul(gmn, gps, 1.0 / GN_COUNT)
                musq = stat_pool.tile([4, 1], F32)
                nc.vector.tensor_mul(musq, gmn[:, 0:1], gmn[:, 0:1])
                murstd = stat_pool.tile([4, 2], F32)
                varv = stat_pool.tile([4, 1], F32)
                nc.vector.tensor_sub(varv, gmn[:, 1:2], musq)
                nc.vector.tensor_scalar_add(varv, varv, EPS)
                # rsqrt on DVE only (bit-hack seed + 3 Newton steps): avoids
                # ACT sqrt/ln table loads that thrash against the exp table.
                yt = stat_pool.tile([4, 1], F32)
                vi = yt.bitcast(I32)
                nc.vector.tensor_scalar(vi, varv.bitcast(I32), 1, None,
                                        op0=OP.arith_shift_right)
                nc.vector.tensor_scalar(vi, vi, -1, 0x5F3759DF,
                                        op0=OP.mult, op1=OP.add)
                for it in range(3):
                    t2 = stat_pool.tile([4, 1], F32, name=f"t2_{it}", tag="t2")
                    nc.vector.tensor_mul(t2, yt, yt)
                    nc.vector.tensor_mul(t2, t2, varv)
                    nc.vector.tensor_scalar(t2, t2, -0.5, 1.5,
                                            op0=OP.mult, op1=OP.add)
                    nc.vector.tensor_mul(
                        murstd[:, 1:2] if it == 2 else yt,
                        yt, t2)
                nc.vector.tensor_copy(murstd[:, 0:1], gmn[:, 0:1])
                cps = tri_ps.tile([P, 2], F32, name="cps", tag="tri")
                nc.tensor.matmul(cps, gmT, murstd, start=True, stop=True)
                ab = ab_pool.tile([P, 2], F32)
                nc.vector.tensor_mul(ab[:, 0:1], wcol, cps[:, 1:2])
                tmpm = stat_pool.tile([P, 1], F32)
                nc.vector.tensor_mul(tmpm, cps[:, 0:1], ab[:, 0:1])
                nc.vector.tensor_sub(ab[:, 1:2], bcol, tmpm)
                return ab

            def gn1_stats_streaming(s):
                """Stats from streamed chunks (emitted early, runs during the
                previous sample's phases; x is re-read at apply time)."""
                abs_ = []
                for ct in range(CT):
                    rows = slice(s * C + ct * P, s * C + (ct + 1) * P)
                    sm8 = stat_pool.tile([P, 8], F32)
                    sq8 = stat_pool.tile([P, 8], F32)
                    for j in range(NCH):
                        xc = xap_pool.tile([P, 512], F32, name="xc", tag="xap")
                        nc.sync.dma_start(xc, x_d[rows, j * 512:(j + 1) * 512])
                        dmp = dump_pool.tile([P, 512], F32)
                        nc.scalar.activation(dmp, xc, AF.Square,
                                             accum_out=sq8[:, j:j + 1])
                        nc.vector.reduce_sum(sm8[:, j:j + 1], xc, axis=AX.X)
                    st2 = stat_pool.tile([P, 2], F32)
                    nc.vector.reduce_sum(st2[:, 0:1], sm8, axis=AX.X)
                    nc.vector.reduce_sum(st2[:, 1:2], sq8, axis=AX.X)
                    abs_.append(gn_params(st2, w1c[:, ct:ct + 1],
                                          b1c[:, ct:ct + 1]))
                return abs_

            seq = [s for _ in range(reps) for s in range(BPC)]
            # sample-0 stats stream first: its x DMAs own the head of the DMA
            # pipe; weight staging (needed only from the first kv matmul at
            # ~25us) follows.
            pending_stats = {0: gn1_stats_streaming(seq[0])}
            for h in range(2):
                for ct in range(CT):
                    stage_round(
                        kvw_sb[:, ct * 2 * C + h * 512: ct * 2 * C + (h + 1) * 512],
                        kvw_d[ct * P:(ct + 1) * P, h * 512:(h + 1) * 512])
            for idx, s in enumerate(seq):
                row0 = s * C
                # ============ GroupNorm 1 apply -> xn tiles ================
                if True:
                    abs_ = pending_stats.pop(idx)
                    xn = []
                    for ct in range(CT):
                        xn.append(x_pool.tile([P, N], F32, name="xnt",
                                              tag="xnt"))
                    # chunk-major applies (x re-read): phase 1 unblocks after
                    # the first column chunk of every ctile
                    for j in range(NCH):
                        for ct in range(CT):
                            rows = slice(row0 + ct * P, row0 + (ct + 1) * P)
                            xc = xap_pool.tile([P, 512], F32, name="xc",
                                               tag="xap")
                            nc.sync.dma_start(
                                xc, x_d[rows, j * 512:(j + 1) * 512])
                            nc.vector.tensor_scalar(
                                _r(xn[ct][:, j * 512:(j + 1) * 512]), xc,
                                abs_[ct][:, 0:1], abs_[ct][:, 1:2],
                                op0=OP.mult, op1=OP.add)

                if idx == 0:
                    # late-needed weights: vb at the ctx drain (~135us), outw
                    # in phase 3 - keep them out of the head DMA window
                    stage_round(vb_row, vb_d, rows=1)
                    for et in range(CT):
                        stage_round(outw_sb[:, et * C:(et + 1) * C],
                                    outw_d[et * P:(et + 1) * P, :])

                # ================= Phase 1: kv^T pass + context =============
                ctx_acc = [quad_ps.tile([P, C], F32, name="ctx_acc", tag="quad") for _ in range(CT)]
                r_row = row_ps.tile([1, C], F32, name="r_row", tag="row")

                def emit_ctx(ekt, vt, nt):
                    nc.tensor.matmul(r_row, _r(ones_col), _r(ekt),
                                     start=(nt == 0), stop=(nt == NT - 1))
                    for dt in range(CT):
                        nc.tensor.matmul(ctx_acc[dt],
                                         _r(ekt[:, dt * P:(dt + 1) * P]), _r(vt),
                                         start=(nt == 0), stop=(nt == NT - 1))

                prev = None
                for nt in range(NT):
                    kps = tri_ps.tile([P, 512], F32, name="kps", tag="tri")
                    for ct in range(CT):
                        nc.tensor.matmul(
                            kps, _r(xn[ct][:, nt * P:(nt + 1) * P]),
                            _r(kvw_sb[:, ct * 2 * C: ct * 2 * C + 512]),
                            start=(ct == 0), stop=(ct == CT - 1))
                    vps = tri_ps.tile([P, 512], F32, name="vps", tag="tri")
                    for ct in range(CT):
                        nc.tensor.matmul(
                            vps, _r(xn[ct][:, nt * P:(nt + 1) * P]),
                            _r(kvw_sb[:, ct * 2 * C + 512: (ct + 1) * 2 * C]),
                            start=(ct == 0), stop=(ct == CT - 1))
                    ekt = ekt_pool.tile([P, 512], F32)
                    nc.scalar.activation(_r(ekt), kps, AF.Exp)  # k-bias cancels
                    vt = vt_pool.tile([P, 512], F32)
                    nc.vector.tensor_copy(_r(vt), vps)      # v-bias folded later
                    if prev is not None:
                        emit_ctx(*prev)
                    prev = (ekt, vt, nt)
                emit_ctx(*prev)

                # R: [1,512] row -> per-partition columns via DRAM bounce
                r_sb = small_pool.tile([1, C], F32, name="r_sb", tag="rcs")
                nc.scalar.copy(r_sb, r_row)
                rb = dram_pool.tile([1, C], F32)
                nc.sync.dma_start(rb, r_sb)
                rcol = small_pool.tile([P, CT], F32)
                nc.sync.dma_start(rcol, rb.rearrange("a (t p) -> (a p) t", p=P))
                rcp = small_pool.tile([P, CT], F32)
                nc.vector.reciprocal(rcp, rcol)
                ctx_sb = []
                for dt in range(CT):
                    t = ctxsb_pool.tile([P, C], F32, name="ctx_sb", tag="ctx_sb")
                    # ctx/R + vb in one DVE op
                    nc.vector.scalar_tensor_tensor(
                        _r(t), ctx_acc[dt], rcp[:, dt:dt + 1], vb_bcast,
                        op0=OP.mult, op1=OP.add)
                    ctx_sb.append(t)
                # next iteration's GN1 stats: emitted here so they run during
                # this sample's phase-2/3 window
                if idx + 1 < len(seq):
                    pending_stats[idx + 1] = gn1_stats_streaming(seq[idx + 1])

                # ================= Phase 2: k pass + attention out ==========
                o2dram = dram_pool.tile([C, N], BF16, name="o2dram", tag="o2dram")
                s2_8 = [stat_pool.tile([P, 8], F32, name="s2_8", tag="s2_8") for _ in range(CT)]
                q2_8 = [stat_pool.tile([P, 8], F32, name="q2_8", tag="q2_8") for _ in range(CT)]
                o2ps = {}
                sps = {}

                def emit_attn(j, dt, ek2):
                    nc.tensor.matmul(sps[j], _r(qinv_col), _r(ek2),
                                     start=(dt == 0), stop=(dt == CT - 1))
                    for et in range(CT):
                        nc.tensor.matmul(o2ps[j][et],
                                         _r(ctx_sb[dt][:, et * P:(et + 1) * P]),
                                         _r(ek2),
                                         start=(dt == 0), stop=(dt == CT - 1))
                    if dt == CT - 1:
                        # drain chunk j: 1/S broadcast, scale, GN2 stats
                        rcs = small_pool.tile([1, 512], F32, name="rcs", tag="rcs")
                        with nc.allow_low_precision(reason="f32r rounding for matmul rhs"):
                            nc.vector.reciprocal(_r(rcs), sps[j][0:1, :])
                        bps = row_ps.tile([P, 512], F32, name="bps", tag="row")
                        nc.tensor.matmul(bps, _r(ones_row), _r(rcs),
                                         start=True, stop=True)
                        bcs = bcs_pool.tile([P, 512], F32)
                        nc.scalar.copy(bcs, bps)
                        for et in range(CT):
                            stg2 = outsb_pool.tile([P, 512], BF16, name="stg2",
                                                   tag="outsb")
                            nc.vector.tensor_mul(stg2, o2ps[j][et], bcs)
                            dmp = dump_pool.tile([P, 512], F32)
                            nc.scalar.activation(dmp, stg2, AF.Square,
                                                 accum_out=q2_8[et][:, j:j + 1])
                            nc.vector.reduce_sum(s2_8[et][:, j:j + 1], stg2,
                                                 axis=AX.X)
                            nc.sync.dma_start(
                                o2dram[et * P:(et + 1) * P,
                                       j * 512:(j + 1) * 512], stg2)
                        del o2ps[j], sps[j]

                pending2 = []
                for j in range(NCH):
                    o2ps[j] = [quad_ps.tile([P, 512], F32, name="o2ps", tag="quad") for _ in range(CT)]
                    sps[j] = row_ps.tile([1, 512], F32, name="sps", tag="row")
                    for dt in range(CT):
                        k2 = tri_ps.tile([P, 512], F32, name="k2", tag="tri")
                        for ct in range(CT):
                            nc.tensor.matmul(
                                k2,
                                _r(kvw_sb[:, ct * 2 * C + dt * P:
                                          ct * 2 * C + (dt + 1) * P]),
                                _r(xn[ct][:, j * 512:(j + 1) * 512]),
                                start=(ct == 0), stop=(ct == CT - 1))
                        ek2 = ek2_pool.tile([P, 512], F32, name="ek2", tag="ek2")
                        nc.scalar.activation(_r(ek2), k2, AF.Exp,
                                             bias=kbc[:, dt:dt + 1])
                        pending2.append((j, dt, ek2))
                        if len(pending2) > 2:
                            emit_attn(*pending2.pop(0))
                for p2 in pending2:
                    emit_attn(*p2)
                # prefetch the gelu ACT table during the phase-2 tail so the
                # GN2->phase3 transition doesn't pay the table load
                gdum = stat_pool.tile([P, 4], F32, name="gdum", tag="gdum")
                nc.scalar.activation(gdum, gm, gelu_f)

                # ================= GroupNorm 2 params =======================
                ab2 = []
                for et in range(CT):
                    st2 = stat_pool.tile([P, 2], F32)
                    nc.vector.reduce_sum(st2[:, 0:1], s2_8[et], axis=AX.X)
                    nc.vector.reduce_sum(st2[:, 1:2], q2_8[et], axis=AX.X)
                    ab2.append(gn_params(st2, w2c[:, et:et + 1], b2c[:, et:et + 1]))

                # ================= Phase 3: gelu + proj + residual ==========
                def emit_proj(j, gts):
                    for ot in range(CT):
                        o3 = quad_ps.tile([P, 512], F32, name="o3", tag="quad")
                        for et in range(CT):
                            nc.tensor.matmul(
                                o3,
                                _r(outw_sb[:, et * C + ot * P: et * C + (ot + 1) * P]),
                                _r(gts[et]),
                                start=(et == 0), stop=(et == CT - 1))
                        ob_sb = outsb_pool.tile([P, 512], F32, name="ob_sb",
                                                 tag="outsb")
                        # (o3 + out_b) + xn in one DVE op
                        nc.vector.scalar_tensor_tensor(
                            ob_sb, o3, obc[:, ot:ot + 1],
                            _r(xn[ot][:, j * 512:(j + 1) * 512]),
                            op0=OP.add, op1=OP.add)
                        nc.sync.dma_start(
                            out_d[row0 + ot * P: row0 + (ot + 1) * P,
                                  j * 512:(j + 1) * 512], ob_sb)

                prev3 = None
                for j in range(NCH):
                    gts = []
                    for et in range(CT):
                        rd = o2rd_pool.tile([P, 512], BF16, name="rd", tag="rd")
                        nc.sync.dma_start(
                            rd, o2dram[et * P:(et + 1) * P,
                                       j * 512:(j + 1) * 512])
                        g = g_pool.tile([P, 512], F32, name="g", tag="g")
                        nc.scalar.activation(_r(g), rd,
                                             gelu_f, bias=ab2[et][:, 1:2],
                                             scale=ab2[et][:, 0:1])
                        gts.append(g)
                    if prev3 is not None:
                        emit_proj(*prev3)
                    prev3 = (j, gts)
                emit_proj(*prev3)

    nc.compile()
    return nc


def prep_inputs(inputs):
    """Host-side prep: shard x over batch, pre-transpose/pack weights."""
    x = np.ascontiguousarray(np.asarray(inputs["x"], dtype=np.float32))
    kv_w = np.asarray(inputs["kv_w"], dtype=np.float32)
    kv_b = np.asarray(inputs["kv_b"], dtype=np.float32)
    out_w = np.asarray(inputs["out_w"], dtype=np.float32)
    out_b = np.asarray(inputs["out_b"], dtype=np.float32)
    w1 = np.asarray(inputs["norm1_w"], dtype=np.float32)
    b1 = np.asarray(inputs["norm1_b"], dtype=np.float32)
    w2 = np.asarray(inputs["norm2_w"], dtype=np.float32)
    b2 = np.asarray(inputs["norm2_b"], dtype=np.float32)

    kvwT = np.ascontiguousarray(kv_w.T)                 # [C, 2C]
    outwT = np.ascontiguousarray(out_w.T)               # [C, C]
    kb = kv_b[:C]
    vb = np.ascontiguousarray(kv_b[C:]).reshape(1, C)
    prm = np.stack([w1, b1, kb, w2, b2, out_b]).reshape(6, CT, P)
    prm = np.ascontiguousarray(prm)
    gmat = np.zeros((P, 4), np.float32)
    for p in range(P):
        gmat[p, p // GSIZE] = 1.0
    gmatT = np.ascontiguousarray(gmat.T)

    xs = x.reshape(B, C, N)
    in_maps = []
    for i in range(N_CORES):
        shard = np.ascontiguousarray(
            xs[i * BPC:(i + 1) * BPC].reshape(BPC * C, N))
        in_maps.append({
            "x": shard, "kvwT": kvwT, "outwT": outwT, "prm": prm,
            "vb": vb, "gmat": gmat, "gmatT": gmatT,
        })
    return in_maps


_NC_CACHE = {}


def get_program(gelu: bool = True, reps: int = 1):
    key = (bool(gelu), reps)
    if key not in _NC_CACHE:
        _NC_CACHE[key] = build_program(gelu=key[0], reps=reps)
    return _NC_CACHE[key]


def run(inputs, trace: bool = False, gelu: bool = True, reps: int = 1):
    """Run on 8 cores; returns (full_output [16,512,64,64], BassKernelResults)."""
    nc = get_program(gelu=gelu, reps=reps)
    in_maps = prep_inputs(inputs)
    res = run_bass_kernel_spmd(nc, in_maps, core_ids=list(range(N_CORES)),
                               trace=trace)
    full = np.empty((B, C, N), np.float32)
    for i in range(N_CORES):
        full[i * BPC:(i + 1) * BPC] = res.results[i]["out"].reshape(BPC, C, N)
    return full.reshape(B, C, H, W), res


def kernel(**inputs) -> np.ndarray:
    out, _ = run(inputs, trace=False, gelu=True)
    return out
